# revision 2
# baseline (speedup 1.0000x reference)
"""Trainium2 Bass kernel for nn_ContrastLoss (supervised prototype contrastive loss).

Strategy (data-parallel over anchors, per the sharding hint):
  * The loss touches `embs` only through a gather of M=4085 feature rows
    (anchor pixels).  That gather is pure data movement, so it is done on the
    host as part of sharding: each of the 8 cores receives its 512-anchor
    shard (anchors padded M -> 4096 = 8*512) in transposed layout [C, 512],
    ready to be used as the matmul stationary operand.
  * The tiny prototype matrix [256, 190] and the per-column masks are
    replicated to every core.  Per-anchor mask rows are derived on-device
    from the anchor labels (1 fused DVE op per 128-anchor chunk).
  * Each core computes mean_log_prob_pos for its 512 anchors plus a per-anchor
    overflow gap ([128, 8] f32 out) and the host finishes with the scalar mean
    over the 4085 real anchors (the "all-reduce" of the hint, host-side on
    8 KB of data).

Device math per 128-anchor chunk (f32 everywhere):
    dot    = anchors_chunk @ protos.T         (PE, 2 matmuls, K=128 each)
    dotm   = dot + maskbias                   (DVE; invalid cols -> -1e30)
    rmax   = rowmax(dotm), gmax = rowmax(dot) (DVE reduces)
    e      = exp(10*dotm - 10*rmax)           (ACT)
    ep     = e * posf                         (DVE)
    en     = e - ep, negsum = sum(en)         (DVE fused; exact, no cancellation)
    d      = ep + negsum + 1e-12              (DVE dual-scalar)
    ld     = ln(d)                            (ACT)
    lp     = 10*dotm - ld                     (DVE fused)
    S1     = sum(posf*lp); out = (S1 + npos*(-10*rmax)) / (npos+1e-12)
which reproduces the reference exactly:  log_prob = logits - log(e*posf +
negsum + 1e-12) summed over positives, with logits = 10*(dot - rowmax_valid).

NaN semantics: the f32 reference computes exp at INVALID columns too
(logits there = dot/T - max_valid/T, which can exceed the f32 exp overflow
threshold 88.722839...), and inf * 0 = NaN then poisons the whole loss.
The kernel exports gap = gmax - rmax per anchor; the host returns NaN iff
any 10*gap crosses the overflow threshold — bit-matching the reference's
NaN behavior without letting inf near the device pipeline.
"""

import numpy as np

import concourse.bass as bass
import concourse.tile as tile
from concourse import bacc, mybir
from concourse.bass_utils import run_bass_kernel_spmd

# Problem geometry (hardcoded per spec: nn_ContrastLoss_11605001634273).
N_CORES = 8
C_FEAT = 256
N_CLASSES = 19
KM = 10
P_PROTO = N_CLASSES * KM          # 190
M_ANCHORS = 4085
M_PAD = 4096                      # 8 cores * 512
M_PER_CORE = M_PAD // N_CORES     # 512
N_CHUNKS = M_PER_CORE // 128      # 4
INV_TEMP = 10.0                   # 1 / TEMPERATURE
NEG_BIG = -1.0e30
# exp(x) overflows to inf in f32 for x >= 88.72283935...; largest finite arg
# is 88.72283172...  Any 10*(gmax-rmax) beyond this makes the reference NaN.
EXP_OVF_THRESHOLD = 88.722835

_CACHE: dict = {}


def _bcast_rows(src: bass.AP, parts: int, n: int) -> bass.AP:
    """DRAM [1, n] -> AP that repeats the row across `parts` partitions."""
    return bass.AP(tensor=src.tensor, offset=src.offset, ap=[[0, parts], [1, n]])


def _body(tc, out, aT, pT, mb, vf, pl, lab, npos, inv):
    nc = tc.nc
    f32 = mybir.dt.float32
    P = P_PROTO
    AluOp = mybir.AluOpType
    Act = mybir.ActivationFunctionType
    X = mybir.AxisListType.X

    with (
        tc.tile_pool(name="const", bufs=1) as const,
        tc.tile_pool(name="work", bufs=3) as work,
        tc.tile_pool(name="acc", bufs=1) as accp,
        tc.tile_pool(name="ps", bufs=4, space="PSUM") as psp,
    ):
        a0 = const.tile([128, M_PER_CORE], f32, tag="a0")
        nc.sync.dma_start(a0, aT[0:128, :])
        a1 = const.tile([128, M_PER_CORE], f32, tag="a1")
        nc.sync.dma_start(a1, aT[128:256, :])
        pt0 = const.tile([128, P], f32, tag="pt0")
        nc.sync.dma_start(pt0, pT[0:128, :])
        pt1 = const.tile([128, P], f32, tag="pt1")
        nc.sync.dma_start(pt1, pT[128:256, :])
        mbr = const.tile([128, P], f32, tag="mbr")
        nc.sync.dma_start(mbr, _bcast_rows(mb, 128, P))
        vfr = const.tile([128, P], f32, tag="vfr")
        nc.sync.dma_start(vfr, _bcast_rows(vf, 128, P))
        plr = const.tile([128, P], f32, tag="plr")
        nc.sync.dma_start(plr, _bcast_rows(pl, 128, P))
        labt = const.tile([128, N_CHUNKS], f32, tag="labt")
        nc.sync.dma_start(labt, lab[:, :])
        npost = const.tile([128, N_CHUNKS], f32, tag="npost")
        nc.sync.dma_start(npost, npos[:, :])
        invt = const.tile([128, N_CHUNKS], f32, tag="invt")
        nc.sync.dma_start(invt, inv[:, :])

        outt = accp.tile([128, 2 * N_CHUNKS], f32, tag="outt")

        for t in range(N_CHUNKS):
            ps = psp.tile([128, P], f32, tag="ps")
            nc.tensor.matmul(ps, a0[:, bass.ts(t, 128)], pt0, start=True, stop=False)
            nc.tensor.matmul(ps, a1[:, bass.ts(t, 128)], pt1, start=False, stop=True)

            # posf = (proto_label == anchor_label) * valid
            posf = work.tile([128, P], f32, tag="posf")
            nc.vector.scalar_tensor_tensor(
                posf, in0=plr, scalar=labt[:, t : t + 1], in1=vfr,
                op0=AluOp.is_equal, op1=AluOp.mult,
            )
            # dotm: invalid columns pushed to -1e30;  rmax = valid row max
            dotm = work.tile([128, P], f32, tag="dotm")
            nc.vector.tensor_tensor(dotm, ps, mbr, op=AluOp.add)
            rmax = work.tile([128, 1], f32, tag="rmax")
            nc.vector.reduce_max(rmax, dotm, axis=X)
            # gmax = unmasked row max (for reference-NaN detection)
            gmax = work.tile([128, 1], f32, tag="gmax")
            nc.vector.reduce_max(gmax, ps, axis=X)
            nc.vector.tensor_tensor(
                outt[:, N_CHUNKS + t : N_CHUNKS + t + 1], gmax, rmax,
                op=AluOp.subtract,
            )
            nbias = work.tile([128, 1], f32, tag="nbias")
            nc.vector.tensor_scalar_mul(nbias, rmax, -INV_TEMP)
            # e = exp(10*dotm + nbias)   (invalid cols -> exp(-huge) = 0)
            e = work.tile([128, P], f32, tag="e")
            nc.scalar.activation(e, dotm, Act.Exp, bias=nbias, scale=INV_TEMP)
            # ep = e * posf ; en = e - ep (exact negatives), negsum = sum(en)
            ep = work.tile([128, P], f32, tag="ep")
            nc.vector.tensor_mul(ep, e, posf)
            en = work.tile([128, P], f32, tag="en")
            negsum = work.tile([128, 1], f32, tag="negsum")
            nc.vector.scalar_tensor_tensor(
                en, in0=e, scalar=1.0, in1=ep,
                op0=AluOp.mult, op1=AluOp.subtract, accum_out=negsum,
            )
            # d = (ep + negsum) + 1e-12 ; ld = ln(d)
            d = work.tile([128, P], f32, tag="d")
            nc.vector.tensor_scalar(
                d, in0=ep, scalar1=negsum, scalar2=1.0e-12,
                op0=AluOp.add, op1=AluOp.add,
            )
            ld = work.tile([128, P], f32, tag="ld")
            nc.scalar.activation(ld, d, Act.Ln)
            # lp = 10*dotm - ld   (log_prob minus the per-row +nbias term)
            lp = work.tile([128, P], f32, tag="lp")
            nc.vector.scalar_tensor_tensor(
                lp, in0=dotm, scalar=INV_TEMP, in1=ld,
                op0=AluOp.mult, op1=AluOp.subtract,
            )
            # s1r = sum(posf * lp);  out = (s1r + npos*nbias) / (npos + 1e-12)
            plp = work.tile([128, P], f32, tag="plp")
            s1r = work.tile([128, 1], f32, tag="s1r")
            nc.vector.scalar_tensor_tensor(
                plp, in0=lp, scalar=1.0, in1=posf,
                op0=AluOp.mult, op1=AluOp.mult, accum_out=s1r,
            )
            nn = work.tile([128, 1], f32, tag="nn")
            nc.vector.tensor_mul(nn, npost[:, t : t + 1], nbias)
            nc.vector.tensor_scalar(
                outt[:, t : t + 1], in0=s1r, scalar1=nn, scalar2=invt[:, t : t + 1],
                op0=AluOp.add, op1=AluOp.mult,
            )

        nc.sync.dma_start(out[:, :], outt)


def _get_program():
    if "nc" not in _CACHE:
        nc = bacc.Bacc(
            "TRN2",
            target_bir_lowering=False,
            debug=False,
            num_devices=N_CORES,
        )
        f32 = mybir.dt.float32
        aT = nc.dram_tensor("aT", [C_FEAT, M_PER_CORE], f32, kind="ExternalInput").ap()
        pT = nc.dram_tensor("pT", [C_FEAT, P_PROTO], f32, kind="ExternalInput").ap()
        mb = nc.dram_tensor("mb", [1, P_PROTO], f32, kind="ExternalInput").ap()
        vf = nc.dram_tensor("vf", [1, P_PROTO], f32, kind="ExternalInput").ap()
        pl = nc.dram_tensor("pl", [1, P_PROTO], f32, kind="ExternalInput").ap()
        lab = nc.dram_tensor("lab", [128, N_CHUNKS], f32, kind="ExternalInput").ap()
        npos = nc.dram_tensor("npos", [128, N_CHUNKS], f32, kind="ExternalInput").ap()
        inv = nc.dram_tensor("inv", [128, N_CHUNKS], f32, kind="ExternalInput").ap()
        out = nc.dram_tensor(
            "out", [128, 2 * N_CHUNKS], f32, kind="ExternalOutput"
        ).ap()
        with tile.TileContext(nc) as tc:
            _body(tc, out, aT, pT, mb, vf, pl, lab, npos, inv)
        nc.compile()
        _CACHE["nc"] = nc
    return _CACHE["nc"]


def _prepare_in_maps(embs, proto_mem, anchor_idx, anchor_labels, proto_mask):
    embs = np.asarray(embs, dtype=np.float32)
    proto_mem = np.asarray(proto_mem, dtype=np.float32)
    anchor_idx = np.asarray(anchor_idx)
    anchor_labels = np.asarray(anchor_labels)
    proto_mask = np.asarray(proto_mask)

    B, C, H, W = embs.shape
    HW = H * W
    n_cls, km, _ = proto_mem.shape
    P = n_cls * km
    M = anchor_idx.shape[0]
    assert C == C_FEAT and P == P_PROTO and M == M_ANCHORS

    idx = anchor_idx.astype(np.int64)
    b = idx // HW
    pix = idx % HW
    # gather anchor feature rows: [M, C]
    anchors = embs.reshape(B, C, HW)[b, :, pix]

    aT_full = np.zeros((C, M_PAD), dtype=np.float32)
    aT_full[:, :M] = anchors.T

    pT = np.ascontiguousarray(proto_mem.reshape(P, C).T)  # [C, P]

    valid = proto_mask.reshape(P).astype(np.float32)
    mbv = np.ascontiguousarray(((valid - 1.0) * 1.0e30).reshape(1, P), np.float32)
    vfv = np.ascontiguousarray(valid.reshape(1, P), np.float32)
    plabels = np.repeat(np.arange(n_cls, dtype=np.float32), km)
    plv = np.ascontiguousarray(plabels.reshape(1, P), np.float32)

    labels = np.full(M_PAD, -1.0, dtype=np.float32)
    labels[:M] = anchor_labels.astype(np.float32)
    cnt = valid.reshape(n_cls, km).sum(axis=1)  # valid protos per class
    nposv = np.zeros(M_PAD, dtype=np.float32)
    nposv[:M] = cnt[anchor_labels.astype(np.int64)]
    invv = (1.0 / (nposv.astype(np.float64) + 1.0e-12)).astype(np.float32)

    def per_core_cols(arr):  # [M_PAD] -> [N_CORES][128, N_CHUNKS]
        a = arr.reshape(N_CORES, N_CHUNKS, 128).transpose(0, 2, 1)
        return [np.ascontiguousarray(a[k]) for k in range(N_CORES)]

    lab_pc = per_core_cols(labels)
    npos_pc = per_core_cols(nposv)
    inv_pc = per_core_cols(invv)

    in_maps = []
    for k in range(N_CORES):
        in_maps.append(
            {
                "aT": np.ascontiguousarray(
                    aT_full[:, k * M_PER_CORE : (k + 1) * M_PER_CORE]
                ),
                "pT": pT,
                "mb": mbv,
                "vf": vfv,
                "pl": plv,
                "lab": lab_pc[k],
                "npos": npos_pc[k],
                "inv": inv_pc[k],
            }
        )
    return in_maps


def _finish(core_outs):
    """core_outs: [N_CORES][128, 2*N_CHUNKS] -> scalar loss (np.float32, ())."""
    outs = np.stack(core_outs)  # [8, 128, 8]
    mlpp = outs[:, :, :N_CHUNKS].transpose(0, 2, 1).reshape(M_PAD)[:M_ANCHORS]
    gaps = outs[:, :, N_CHUNKS:].transpose(0, 2, 1).reshape(M_PAD)[:M_ANCHORS]
    if np.any(gaps.astype(np.float64) * INV_TEMP > EXP_OVF_THRESHOLD):
        # f32 reference overflows exp at an invalid column -> inf*0 -> NaN loss
        return np.asarray(np.nan, dtype=np.float32)
    loss = -(mlpp.astype(np.float64).sum() / M_ANCHORS)
    return np.asarray(loss, dtype=np.float32)


def kernel(embs, proto_mem, anchor_idx, anchor_labels, proto_mask):
    in_maps = _prepare_in_maps(embs, proto_mem, anchor_idx, anchor_labels, proto_mask)
    nc = _get_program()
    res = run_bass_kernel_spmd(nc, in_maps, core_ids=list(range(N_CORES)))
    return _finish([res.results[k]["out"] for k in range(N_CORES)])


def kernel_sim(embs, proto_mem, anchor_idx, anchor_labels, proto_mask):
    """CoreSim-based functional check (no hardware). Dev/debug only."""
    from concourse.bass_interp import CoreSim

    in_maps = _prepare_in_maps(embs, proto_mem, anchor_idx, anchor_labels, proto_mask)
    nc = _get_program()
    core_outs = []
    for k in range(N_CORES):
        sim = CoreSim(nc, trace=False)
        for name, arr in in_maps[k].items():
            sim.tensor(name)[:] = arr
        sim.simulate(check_with_hw=False)
        core_outs.append(np.array(sim.tensor("out")))
    return _finish(core_outs)
